# revision 19
# baseline (speedup 1.0000x reference)
"""AllPairs triu kernel for Trainium2 (8 NeuronCores, one molecule per core).

Computes, for each molecule of N=2048 atoms, all upper-triangle pairs:
  indices [2, P] int32 (flat atom ids, -1 where invalid)
  dist    [P]    f32   (||ri - rj|| where valid else 0)
  diff    [P, 3] f32   (ri - rj where valid else 0)
with P = B * N*(N-1)/2, valid = both atoms real and dist <= 5.2.

Strategy: data-parallel over the batch axis (1 molecule per core). Each core
computes 128-row i-tiles against the trimmed j-range [128t, N), then writes
the packed triangular output directly with indirect-scatter DMAs:
  - main chunk per row: constant length C_t = 1920-128t starting at the row's
    packed base (diagonal source access pattern, row p starts at column p+1);
  - one backward window per tile: each row's last 127 elements, which are
    always the columns j in [1921, 2048) (clean access pattern); overlap with
    the main chunk rewrites identical bytes, which is benign.
The i-tile t=15 (row lengths < 128, total 8128 pairs = 0.4% of the output)
is emitted as a small dense block and packed on the host.
The dist<=cutoff mask is computed exactly as sqrt-compare via an equivalent
dist^2 threshold, so masks match the IEEE reference bit-for-bit.
"""
import sys

sys.path.insert(0, "/opt/trn_rl_repo")

import numpy as np

N = 2048
B = 8
NT = 16                  # i-tiles per molecule
PM = N * (N - 1) // 2    # pairs per molecule = 2096128
CUTOFF = 5.2
NCOLS = 32               # offset-table columns per tensor


def _exact_d2_threshold():
    """Largest f32 u with np.sqrt(f32(u)) <= f32(5.2).

    sqrt is monotone and IEEE-correctly-rounded on both CPU (reference) and in
    this comparison, so (d2 <= U) == (sqrt(d2) <= 5.2f) for every f32 d2.
    """
    c = np.float32(CUTOFF)
    lo_bits = (c * c).view(np.uint32)
    while np.sqrt(lo_bits.view(np.float32)) > c:
        lo_bits = np.uint32(lo_bits - 1)
    hi_bits = np.uint32(lo_bits + 1000)
    assert np.sqrt(hi_bits.view(np.float32)) > c
    lo_bits, hi_bits = int(lo_bits), int(hi_bits)
    while hi_bits - lo_bits > 1:
        mid = (lo_bits + hi_bits) // 2
        if np.sqrt(np.uint32(mid).view(np.float32)) <= c:
            lo_bits = mid
        else:
            hi_bits = mid
    return float(np.uint32(lo_bits).view(np.float32))


U_D2 = _exact_d2_threshold()


def _base(i):
    """Packed offset of row i in the per-molecule triu output."""
    return 2047 * i - (i * (i - 1)) // 2


def _build_offset_table():
    """[128, NCOLS] int64: cols 0..14 mains(t): row p covers columns
    j in [128(t+1), 2048), i.e. packed [base(i) + 127 - p, base(i) + L)."""
    tab = np.zeros((128, NCOLS), dtype=np.int64)
    p = np.arange(128)
    for t in range(15):
        i = 128 * t + p
        tab[:, t] = _base(i) + 127 - p
    return tab


_OFFSET_TABLE = _build_offset_table()


def _tensor_tables():
    t = _OFFSET_TABLE
    mk = lambda s, o: (s * t + o).astype(np.int32)
    return {"tabsall": np.concatenate(
        [mk(1, 0), mk(1, 0), mk(1, PM), mk(3, 0)], axis=1)}


# ----------------------------------------------------------------------------
# Bass graph
# ----------------------------------------------------------------------------

_GRAPH = None


def _build_graph():
    import concourse.bass as bass
    import concourse.bacc as bacc
    import concourse.mybir as mybir
    from concourse.tile import TileContext

    OP = mybir.AluOpType
    AF = mybir.ActivationFunctionType
    f32 = mybir.dt.float32
    i32 = mybir.dt.int32

    nc = bacc.Bacc("TRN2", debug=False, num_devices=B, detect_race_conditions=False,
                   dynamic_dma_scratch_size=8192)

    negb3 = nc.dram_tensor("negb3", [128, 3 * N], f32, kind="ExternalInput")
    jrow = nc.dram_tensor("jrow", [128, N], f32, kind="ExternalInput")
    # combined small input: cols [0,64) = xyzi+ibias (f32 bits), [64,192) = tables
    xibtab = nc.dram_tensor("xibtab", [128, 4 * NT + 4 * NCOLS], i32, kind="ExternalInput")

    indices_o = nc.dram_tensor("indices", [2, PM], i32, kind="ExternalOutput")
    dist_o = nc.dram_tensor("dist", [PM], f32, kind="ExternalOutput")
    diff_o = nc.dram_tensor("diff", [PM, 3], f32, kind="ExternalOutput")
    # per-tile diagonal 128x128 blocks, packed on the host
    blkd = nc.dram_tensor("blkd", [NT * 128, 128], f32, kind="ExternalOutput")
    blki0 = nc.dram_tensor("blki0", [NT * 128, 128], i32, kind="ExternalOutput")
    blki1 = nc.dram_tensor("blki1", [NT * 128, 128], i32, kind="ExternalOutput")
    blkD = nc.dram_tensor("blkD", [NT * 128, 384], f32, kind="ExternalOutput")

    ind_flat = indices_o[:].flatten().unsqueeze(1)
    dist_flat = dist_o[:].unsqueeze(1)
    diff_flat = diff_o[:].flatten().unsqueeze(1)

    out_dma_names = set()

    def _strip_outdma_deps(bi):
        """Output DMAs overlap only where bytes are identical; drop the WAW
        serialization Tile would impose so they pipeline."""
        ins = bi.ins
        for d in list(ins.sync_dependency_names()):
            if d in out_dma_names:
                ins.try_remove_dependency(d)
        for d in list(ins.nosync_dependency_names()):
            if d in out_dma_names:
                ins.try_remove_dependency(d)
        out_dma_names.add(ins.name)
        return bi

    with TileContext(nc) as tc:
        tc.race_detector_enabled = False
        with (
            tc.tile_pool(name="persist", bufs=1) as pp,
            tc.tile_pool(name="outs", bufs=2) as po,
            tc.tile_pool(name="dxyzp", bufs=2) as pd,
            tc.tile_pool(name="sqa", bufs=2) as ps,
            tc.tile_pool(name="sqbc", bufs=1) as psc,
        ):
            t_negb3 = pp.tile([128, 3 * N], f32, tag="negb3")
            for c in range(3):
                nc.sync.dma_start(out=t_negb3[:, c * N : (c + 1) * N],
                                  in_=negb3[:, c * N : (c + 1) * N])
            t_jrow = pp.tile([128, N], f32, tag="jrow")
            nc.sync.dma_start(out=t_jrow[:], in_=jrow[:])
            t_comb = pp.tile([128, 4 * NT + 4 * NCOLS], i32, tag="xibtab")
            nc.sync.dma_start(out=t_comb[:], in_=xibtab[:])
            TABW = 4 * NT + 4 * NCOLS
            TAB0 = 4 * NT

            def scatter(dst_flat, tab_off, src_tile, src_off, src_dims):
                toff = bass.AP(t_comb[:].tensor, TAB0 + tab_off, [[TABW, 128], [1, 1]])
                src = bass.AP(src_tile[:].tensor, src_off, src_dims)
                bi = nc.gpsimd.indirect_dma_start(
                    out=dst_flat,
                    out_offset=bass.IndirectOffsetOnAxis(ap=toff, axis=0),
                    in_=src,
                    in_offset=None,
                )
                return _strip_outdma_deps(bi)

            def phase_a(t):
                W = N - 128 * t
                t_diff = po.tile([128, 3 * N], f32, tag="diff", name=f"diff{t}")
                t_dist = po.tile([128, N], f32, tag="dist", name=f"dist{t}")
                t_idx0 = po.tile([128, N], i32, tag="idx0", name=f"idx0_{t}")
                t_idx1 = po.tile([128, N], i32, tag="idx1", name=f"idx1_{t}")
                dxyz = pd.tile([128, 3 * N], f32, tag="dxyz", name=f"dxyz{t}")
                sq_a = ps.tile([128, N], f32, tag="sq_a", name=f"sqa{t}")
                sq_b = psc.tile([128, N], f32, tag="sq_b", name=f"sqb{t}")
                sq_c = psc.tile([128, N], f32, tag="sq_c", name=f"sqc{t}")

                # ACT: diff components, contiguous blocks (exact IEEE adds)
                for c in range(3):
                    nc.scalar.activation(
                        out=dxyz[:, c * N : c * N + W],
                        in_=t_negb3[:, c * N + 128 * t : (c + 1) * N],
                        func=AF.Identity,
                        bias=t_comb[:, 3 * t + c : 3 * t + c + 1].bitcast(f32),
                        scale=1.0,
                    )
                # ACT: squares, contiguous (bit-exact)
                nc.scalar.activation(out=sq_a[:, :W], in_=dxyz[:, 0:W], func=AF.Square)
                nc.scalar.activation(out=sq_b[:, :W], in_=dxyz[:, N : N + W], func=AF.Square)
                nc.scalar.activation(out=sq_c[:, :W], in_=dxyz[:, 2 * N : 2 * N + W], func=AF.Square)
                # DVE: d2 = sqx + sqy + sqz ; mask ; d2m ; masked interleave
                nc.vector.tensor_tensor(out=sq_a[:, :W], in0=sq_a[:, :W], in1=sq_b[:, :W], op=OP.add)
                nc.vector.tensor_tensor(out=sq_a[:, :W], in0=sq_a[:, :W], in1=sq_c[:, :W], op=OP.add)
                nc.gpsimd.tensor_scalar(out=sq_b[:, :W], in0=sq_a[:, :W],
                                        scalar1=U_D2, scalar2=None, op0=OP.is_le)
                nc.vector.tensor_tensor(out=sq_a[:, :W], in0=sq_b[:, :W], in1=sq_a[:, :W], op=OP.mult)
                # diff[:, 3j+c] = dxyz[c-block][j] * mask[j]  (one op, [W,3] view)
                dsrc = bass.AP(dxyz[:].tensor, 0, [[3 * N, 128], [1, W], [N, 3]])
                m3 = bass.AP(sq_b[:].tensor, 0, [[N, 128], [1, W], [0, 3]])
                dout = bass.AP(t_diff[:].tensor, 0, [[3 * N, 128], [3, W], [1, 3]])
                nc.vector.tensor_tensor(out=dout, in0=dsrc, in1=m3, op=OP.mult)
                nc.gpsimd.tensor_scalar(out=t_idx0[:, :W], in0=sq_b[:, :W],
                                        scalar1=t_comb[:, 48 + t : 48 + t + 1].bitcast(f32), scalar2=-1.0,
                                        op0=OP.mult, op1=OP.add)
                nc.vector.scalar_tensor_tensor(out=t_idx1[:, :W],
                                               in0=t_jrow[:, 128 * t :], scalar=1.0,
                                               in1=sq_b[:, :W], op0=OP.mult, op1=OP.mult)
                nc.gpsimd.tensor_scalar(out=t_idx1[:, :W], in0=t_idx1[:, :W],
                                        scalar1=-1.0, scalar2=None, op0=OP.add)
                return dict(t=t, W=W, diff=t_diff, dist=t_dist, idx0=t_idx0,
                            idx1=t_idx1, sq_a=sq_a, sq_b=sq_b)

            def phase_b(st):
                t, W = st["t"], st["W"]
                C = 1920 - 128 * t
                t_diff, t_dist = st["diff"], st["dist"]
                t_idx0, t_idx1 = st["idx0"], st["idx1"]
                sq_a, sq_b = st["sq_a"], st["sq_b"]
                nc.scalar.activation(out=t_dist[:, :W], in_=sq_a[:, :W], func=AF.Sqrt)
                if t < 15:
                    # mains: every row sources local cols [128, W);
                    # dest = base(i) + 127 - p, length C = W - 128
                    scatter(dist_flat, t, t_dist, 128, [[N, 128], [1, C]])
                    scatter(ind_flat, NCOLS + t, t_idx0, 128, [[N, 128], [1, C]])
                    scatter(ind_flat, 2 * NCOLS + t, t_idx1, 128, [[N, 128], [1, C]])
                    scatter(diff_flat, 3 * NCOLS + t, t_diff, 384, [[3 * N, 128], [1, 3 * C]])
                # diagonal block dump (cols [0, 128)), host-packed
                for dst, tile_, w in ((blkd, t_dist, 128), (blki0, t_idx0, 128),
                                      (blki1, t_idx1, 128), (blkD, t_diff, 384)):
                    _strip_outdma_deps(nc.sync.dma_start(
                        out=dst[128 * t : 128 * (t + 1), :], in_=tile_[:, :w]))

            # software pipeline: phase B lags one tile behind phase A
            prev = phase_a(0)
            for t in range(1, NT):
                cur = phase_a(t)
                phase_b(prev)
                prev = cur
            phase_b(prev)

    nc.compile()
    return nc


def _get_graph():
    global _GRAPH
    if _GRAPH is None:
        _GRAPH = _build_graph()
    return _GRAPH


# ----------------------------------------------------------------------------
# Host glue
# ----------------------------------------------------------------------------

# diagonal-block host packing: packed positions and dense-block sources
_BLK_P, _BLK_C = np.triu_indices(128, 1)
_BLK_DST = np.concatenate([
    _base(128 * t + _BLK_P) + (_BLK_C - _BLK_P - 1) for t in range(NT)
]).astype(np.int64)
_BLK_SRC = np.concatenate([
    (128 * t + _BLK_P) * 128 + _BLK_C for t in range(NT)
]).astype(np.int64)
_BLK_SRC3 = np.concatenate([
    (128 * t + _BLK_P) * 384 + 3 * _BLK_C for t in range(NT)
]).astype(np.int64)


def _prep_core_inputs(b, coords_mod, tables):
    cb = coords_mod[b]  # [N, 3] f32
    negrow = (-cb.T).reshape(1, 3 * N)
    negb3 = np.ascontiguousarray(np.broadcast_to(negrow, (128, 3 * N)), dtype=np.float32)
    jr = (np.arange(N, dtype=np.float32) + np.float32(b * N + 1)).reshape(1, N)
    jrow = np.ascontiguousarray(np.broadcast_to(jr, (128, N)), dtype=np.float32)
    xyzi = cb.reshape(NT, 128, 3).transpose(1, 0, 2).reshape(128, 3 * NT)
    p = np.arange(128, dtype=np.float32).reshape(128, 1)
    tgrid = np.arange(NT, dtype=np.float32).reshape(1, NT)
    ibias = (128.0 * tgrid + p) + np.float32(b * N + 1)
    xibias = np.concatenate([xyzi, ibias], axis=1).astype(np.float32)
    xibtab = np.ascontiguousarray(np.concatenate(
        [xibias.view(np.int32), tables["tabsall"]], axis=1))
    return {"negb3": negb3, "jrow": jrow, "xibtab": xibtab}


def kernel(species, coords, _trace=False):
    from concourse.bass_utils import run_bass_kernel_spmd

    species = np.asarray(species)
    coords = np.ascontiguousarray(np.asarray(coords, dtype=np.float32))
    assert coords.shape == (B, N, 3), coords.shape

    coords_mod = coords
    dummy = species == -1  # [B, N]
    if dummy.any():
        far = (1e6 * (np.arange(N, dtype=np.float32) + 1.0))[None, :, None]
        far = np.broadcast_to(far, coords.shape)
        coords_mod = np.where(dummy[:, :, None], far, coords).astype(np.float32)

    tables = _tensor_tables()
    nc = _get_graph()
    in_maps = [_prep_core_inputs(b, coords_mod, tables) for b in range(B)]
    res = run_bass_kernel_spmd(nc, in_maps, core_ids=list(range(B)), trace=_trace)

    outs = []
    for c in range(B):
        r = res.results[c]
        ind = np.array(r["indices"])
        dist = np.array(r["dist"])
        diff = np.array(r["diff"]).reshape(PM, 3)
        # host-pack the 16 diagonal-block triangles (6.2% of pairs)
        ind[0, _BLK_DST] = r["blki0"].reshape(-1)[_BLK_SRC]
        ind[1, _BLK_DST] = r["blki1"].reshape(-1)[_BLK_SRC]
        dist[_BLK_DST] = r["blkd"].reshape(-1)[_BLK_SRC]
        dD = r["blkD"].reshape(-1)
        for cmp_ in range(3):
            diff[_BLK_DST, cmp_] = dD[_BLK_SRC3 + cmp_]
        outs.append((ind, dist, diff))

    indices = np.concatenate([o[0] for o in outs], axis=1)
    dist = np.concatenate([o[1] for o in outs], axis=0)
    diff = np.concatenate([o[2] for o in outs], axis=0)
    if _trace:
        kernel.last_exec_time_ns = res.exec_time_ns
        kernel.last_results = res
    return indices, dist, diff


# revision 20
# speedup vs baseline: 3.0663x; 3.0663x over previous
"""AllPairs triu kernel for Trainium2 (8 NeuronCores, one molecule per core).

Computes, for each molecule of N=2048 atoms, all upper-triangle pairs:
  indices [2, P] int32 (flat atom ids, -1 where invalid)
  dist    [P]    f32   (||ri - rj|| where valid else 0)
  diff    [P, 3] f32   (ri - rj where valid else 0)
with P = B * N*(N-1)/2, valid = both atoms real and dist <= 5.2.

Strategy: data-parallel over the batch axis (1 molecule per core). Each core
computes 128-row i-tiles against the trimmed j-range [128t, N), then writes
the packed triangular output directly with indirect-scatter DMAs:
  - main chunk per row: constant length C_t = 1920-128t starting at the row's
    packed base (diagonal source access pattern, row p starts at column p+1);
  - one backward window per tile: each row's last 127 elements, which are
    always the columns j in [1921, 2048) (clean access pattern); overlap with
    the main chunk rewrites identical bytes, which is benign.
The i-tile t=15 (row lengths < 128, total 8128 pairs = 0.4% of the output)
is emitted as a small dense block and packed on the host.
The dist<=cutoff mask is computed exactly as sqrt-compare via an equivalent
dist^2 threshold, so masks match the IEEE reference bit-for-bit.
"""
import sys

sys.path.insert(0, "/opt/trn_rl_repo")

import numpy as np

N = 2048
B = 8
NT = 16                  # i-tiles per molecule
PM = N * (N - 1) // 2    # pairs per molecule = 2096128
CUTOFF = 5.2
NCOLS = 32               # offset-table columns per tensor


def _exact_d2_threshold():
    """Largest f32 u with np.sqrt(f32(u)) <= f32(5.2).

    sqrt is monotone and IEEE-correctly-rounded on both CPU (reference) and in
    this comparison, so (d2 <= U) == (sqrt(d2) <= 5.2f) for every f32 d2.
    """
    c = np.float32(CUTOFF)
    lo_bits = (c * c).view(np.uint32)
    while np.sqrt(lo_bits.view(np.float32)) > c:
        lo_bits = np.uint32(lo_bits - 1)
    hi_bits = np.uint32(lo_bits + 1000)
    assert np.sqrt(hi_bits.view(np.float32)) > c
    lo_bits, hi_bits = int(lo_bits), int(hi_bits)
    while hi_bits - lo_bits > 1:
        mid = (lo_bits + hi_bits) // 2
        if np.sqrt(np.uint32(mid).view(np.float32)) <= c:
            lo_bits = mid
        else:
            hi_bits = mid
    return float(np.uint32(lo_bits).view(np.float32))


U_D2 = _exact_d2_threshold()


def _base(i):
    """Packed offset of row i in the per-molecule triu output."""
    return 2047 * i - (i * (i - 1)) // 2


def _build_offset_table():
    """[128, NCOLS] int64: cols 0..14 mains(t): row p covers columns
    j in [128(t+1), 2048), i.e. packed [base(i) + 127 - p, base(i) + L)."""
    tab = np.zeros((128, NCOLS), dtype=np.int64)
    p = np.arange(128)
    for t in range(15):
        i = 128 * t + p
        tab[:, t] = _base(i) + 127 - p
    return tab


_OFFSET_TABLE = _build_offset_table()


def _tensor_tables():
    t = _OFFSET_TABLE
    mk = lambda s, o: (s * t + o).astype(np.int32)
    return {"tabsall": np.concatenate(
        [mk(1, 0), mk(1, 0), mk(1, PM), mk(3, 0)], axis=1)}


# ----------------------------------------------------------------------------
# Bass graph
# ----------------------------------------------------------------------------

_GRAPH = None


def _build_graph():
    import concourse.bass as bass
    import concourse.bacc as bacc
    import concourse.mybir as mybir
    from concourse.tile import TileContext

    OP = mybir.AluOpType
    AF = mybir.ActivationFunctionType
    f32 = mybir.dt.float32
    i32 = mybir.dt.int32

    nc = bacc.Bacc("TRN2", debug=False, num_devices=B, detect_race_conditions=False,
                   dynamic_dma_scratch_size=8192)

    negb3 = nc.dram_tensor("negb3", [128, 3 * N], f32, kind="ExternalInput")
    jrow = nc.dram_tensor("jrow", [128, N], f32, kind="ExternalInput")
    # combined small input: cols [0,64) = xyzi+ibias (f32 bits), [64,192) = tables
    xibtab = nc.dram_tensor("xibtab", [128, 4 * NT + 4 * NCOLS], i32, kind="ExternalInput")

    indices_o = nc.dram_tensor("indices", [2, PM], i32, kind="ExternalOutput")
    dist_o = nc.dram_tensor("dist", [PM], f32, kind="ExternalOutput")
    diff_o = nc.dram_tensor("diff", [PM, 3], f32, kind="ExternalOutput")
    # per-tile diagonal 128x128 blocks, packed on the host
    blkd = nc.dram_tensor("blkd", [NT * 128, 128], f32, kind="ExternalOutput")
    blki0 = nc.dram_tensor("blki0", [NT * 128, 128], i32, kind="ExternalOutput")
    blki1 = nc.dram_tensor("blki1", [NT * 128, 128], i32, kind="ExternalOutput")
    blkD = nc.dram_tensor("blkD", [NT * 128, 384], f32, kind="ExternalOutput")

    ind_flat = indices_o[:].flatten().unsqueeze(1)
    dist_flat = dist_o[:].unsqueeze(1)
    diff_flat = diff_o[:].flatten().unsqueeze(1)

    out_dma_names = set()

    def _strip_outdma_deps(bi):
        """Output DMAs overlap only where bytes are identical; drop the WAW
        serialization Tile would impose so they pipeline."""
        ins = bi.ins
        for d in list(ins.sync_dependency_names()):
            if d in out_dma_names:
                ins.try_remove_dependency(d)
        for d in list(ins.nosync_dependency_names()):
            if d in out_dma_names:
                ins.try_remove_dependency(d)
        out_dma_names.add(ins.name)
        return bi

    with TileContext(nc) as tc:
        tc.race_detector_enabled = False
        with (
            tc.tile_pool(name="persist", bufs=1) as pp,
            tc.tile_pool(name="outs", bufs=2) as po,
            tc.tile_pool(name="dxyzp", bufs=2) as pd,
            tc.tile_pool(name="sqa", bufs=2) as ps,
            tc.tile_pool(name="sqbc", bufs=1) as psc,
        ):
            t_negb3 = pp.tile([128, 3 * N], f32, tag="negb3")
            for c in range(3):
                nc.sync.dma_start(out=t_negb3[:, c * N : (c + 1) * N],
                                  in_=negb3[:, c * N : (c + 1) * N])
            t_jrow = pp.tile([128, N], f32, tag="jrow")
            nc.sync.dma_start(out=t_jrow[:], in_=jrow[:])
            t_comb = pp.tile([128, 4 * NT + 4 * NCOLS], i32, tag="xibtab")
            nc.sync.dma_start(out=t_comb[:], in_=xibtab[:])
            TABW = 4 * NT + 4 * NCOLS
            TAB0 = 4 * NT

            def scatter(dst_flat, tab_off, src_tile, src_off, src_dims):
                toff = bass.AP(t_comb[:].tensor, TAB0 + tab_off, [[TABW, 128], [1, 1]])
                src = bass.AP(src_tile[:].tensor, src_off, src_dims)
                bi = nc.gpsimd.indirect_dma_start(
                    out=dst_flat,
                    out_offset=bass.IndirectOffsetOnAxis(ap=toff, axis=0),
                    in_=src,
                    in_offset=None,
                )
                return _strip_outdma_deps(bi)

            def phase_a(t):
                W = N - 128 * t
                t_diff = po.tile([128, 3 * N], f32, tag="diff", name=f"diff{t}")
                t_dist = po.tile([128, N], f32, tag="dist", name=f"dist{t}")
                t_idx0 = po.tile([128, N], i32, tag="idx0", name=f"idx0_{t}")
                t_idx1 = po.tile([128, N], i32, tag="idx1", name=f"idx1_{t}")
                dxyz = pd.tile([128, 3 * N], f32, tag="dxyz", name=f"dxyz{t}")
                sq_a = ps.tile([128, N], f32, tag="sq_a", name=f"sqa{t}")
                sq_b = psc.tile([128, N], f32, tag="sq_b", name=f"sqb{t}")
                sq_c = psc.tile([128, N], f32, tag="sq_c", name=f"sqc{t}")

                # ACT: diff components, contiguous blocks (exact IEEE adds)
                for c in range(3):
                    nc.scalar.activation(
                        out=dxyz[:, c * N : c * N + W],
                        in_=t_negb3[:, c * N + 128 * t : (c + 1) * N],
                        func=AF.Identity,
                        bias=t_comb[:, 3 * t + c : 3 * t + c + 1].bitcast(f32),
                        scale=1.0,
                    )
                # ACT: squares, contiguous (bit-exact)
                nc.scalar.activation(out=sq_a[:, :W], in_=dxyz[:, 0:W], func=AF.Square)
                nc.scalar.activation(out=sq_b[:, :W], in_=dxyz[:, N : N + W], func=AF.Square)
                nc.scalar.activation(out=sq_c[:, :W], in_=dxyz[:, 2 * N : 2 * N + W], func=AF.Square)
                # DVE: d2 = sqx + sqy + sqz ; mask ; d2m ; masked interleave
                nc.vector.tensor_tensor(out=sq_a[:, :W], in0=sq_a[:, :W], in1=sq_b[:, :W], op=OP.add)
                nc.vector.tensor_tensor(out=sq_a[:, :W], in0=sq_a[:, :W], in1=sq_c[:, :W], op=OP.add)
                nc.vector.tensor_scalar(out=sq_b[:, :W], in0=sq_a[:, :W],
                                        scalar1=U_D2, scalar2=None, op0=OP.is_le)
                nc.vector.tensor_tensor(out=sq_a[:, :W], in0=sq_b[:, :W], in1=sq_a[:, :W], op=OP.mult)
                # diff[:, 3j+c] = dxyz[c-block][j] * mask[j]
                for c in range(3):
                    dout = bass.AP(t_diff[:].tensor, c, [[3 * N, 128], [3, W]])
                    nc.vector.tensor_tensor(out=dout, in0=dxyz[:, c * N : c * N + W],
                                            in1=sq_b[:, :W], op=OP.mult)
                nc.scalar.activation(out=t_idx0[:, :W], in_=sq_b[:, :W], func=AF.Copy,
                                     bias=-1.0, scale=t_comb[:, 48 + t : 48 + t + 1].bitcast(f32))
                nc.vector.scalar_tensor_tensor(out=t_idx1[:, :W],
                                               in0=t_jrow[:, 128 * t :], scalar=1.0,
                                               in1=sq_b[:, :W], op0=OP.mult, op1=OP.mult)
                nc.scalar.activation(out=t_idx1[:, :W], in_=t_idx1[:, :W], func=AF.Copy,
                                     bias=-1.0, scale=1.0)
                return dict(t=t, W=W, diff=t_diff, dist=t_dist, idx0=t_idx0,
                            idx1=t_idx1, sq_a=sq_a, sq_b=sq_b)

            def phase_b(st):
                t, W = st["t"], st["W"]
                C = 1920 - 128 * t
                t_diff, t_dist = st["diff"], st["dist"]
                t_idx0, t_idx1 = st["idx0"], st["idx1"]
                sq_a, sq_b = st["sq_a"], st["sq_b"]
                nc.scalar.activation(out=t_dist[:, :W], in_=sq_a[:, :W], func=AF.Sqrt)
                if t < 15:
                    # mains: every row sources local cols [128, W);
                    # dest = base(i) + 127 - p, length C = W - 128
                    scatter(dist_flat, t, t_dist, 128, [[N, 128], [1, C]])
                    scatter(ind_flat, NCOLS + t, t_idx0, 128, [[N, 128], [1, C]])
                    scatter(ind_flat, 2 * NCOLS + t, t_idx1, 128, [[N, 128], [1, C]])
                    scatter(diff_flat, 3 * NCOLS + t, t_diff, 384, [[3 * N, 128], [1, 3 * C]])
                # diagonal block dump (cols [0, 128)), host-packed
                for dst, tile_, w in ((blkd, t_dist, 128), (blki0, t_idx0, 128),
                                      (blki1, t_idx1, 128), (blkD, t_diff, 384)):
                    _strip_outdma_deps(nc.sync.dma_start(
                        out=dst[128 * t : 128 * (t + 1), :], in_=tile_[:, :w]))

            # software pipeline: phase B lags one tile behind phase A
            prev = phase_a(0)
            for t in range(1, NT):
                cur = phase_a(t)
                phase_b(prev)
                prev = cur
            phase_b(prev)

    nc.compile()
    return nc


def _get_graph():
    global _GRAPH
    if _GRAPH is None:
        _GRAPH = _build_graph()
    return _GRAPH


# ----------------------------------------------------------------------------
# Host glue
# ----------------------------------------------------------------------------

# diagonal-block host packing: packed positions and dense-block sources
_BLK_P, _BLK_C = np.triu_indices(128, 1)
_BLK_DST = np.concatenate([
    _base(128 * t + _BLK_P) + (_BLK_C - _BLK_P - 1) for t in range(NT)
]).astype(np.int64)
_BLK_SRC = np.concatenate([
    (128 * t + _BLK_P) * 128 + _BLK_C for t in range(NT)
]).astype(np.int64)
_BLK_SRC3 = np.concatenate([
    (128 * t + _BLK_P) * 384 + 3 * _BLK_C for t in range(NT)
]).astype(np.int64)


def _prep_core_inputs(b, coords_mod, tables):
    cb = coords_mod[b]  # [N, 3] f32
    negrow = (-cb.T).reshape(1, 3 * N)
    negb3 = np.ascontiguousarray(np.broadcast_to(negrow, (128, 3 * N)), dtype=np.float32)
    jr = (np.arange(N, dtype=np.float32) + np.float32(b * N + 1)).reshape(1, N)
    jrow = np.ascontiguousarray(np.broadcast_to(jr, (128, N)), dtype=np.float32)
    xyzi = cb.reshape(NT, 128, 3).transpose(1, 0, 2).reshape(128, 3 * NT)
    p = np.arange(128, dtype=np.float32).reshape(128, 1)
    tgrid = np.arange(NT, dtype=np.float32).reshape(1, NT)
    ibias = (128.0 * tgrid + p) + np.float32(b * N + 1)
    xibias = np.concatenate([xyzi, ibias], axis=1).astype(np.float32)
    xibtab = np.ascontiguousarray(np.concatenate(
        [xibias.view(np.int32), tables["tabsall"]], axis=1))
    return {"negb3": negb3, "jrow": jrow, "xibtab": xibtab}


def kernel(species, coords, _trace=False):
    from concourse.bass_utils import run_bass_kernel_spmd

    species = np.asarray(species)
    coords = np.ascontiguousarray(np.asarray(coords, dtype=np.float32))
    assert coords.shape == (B, N, 3), coords.shape

    coords_mod = coords
    dummy = species == -1  # [B, N]
    if dummy.any():
        far = (1e6 * (np.arange(N, dtype=np.float32) + 1.0))[None, :, None]
        far = np.broadcast_to(far, coords.shape)
        coords_mod = np.where(dummy[:, :, None], far, coords).astype(np.float32)

    tables = _tensor_tables()
    nc = _get_graph()
    in_maps = [_prep_core_inputs(b, coords_mod, tables) for b in range(B)]
    res = run_bass_kernel_spmd(nc, in_maps, core_ids=list(range(B)), trace=_trace)

    outs = []
    for c in range(B):
        r = res.results[c]
        ind = np.array(r["indices"])
        dist = np.array(r["dist"])
        diff = np.array(r["diff"]).reshape(PM, 3)
        # host-pack the 16 diagonal-block triangles (6.2% of pairs)
        ind[0, _BLK_DST] = r["blki0"].reshape(-1)[_BLK_SRC]
        ind[1, _BLK_DST] = r["blki1"].reshape(-1)[_BLK_SRC]
        dist[_BLK_DST] = r["blkd"].reshape(-1)[_BLK_SRC]
        dD = r["blkD"].reshape(-1)
        for cmp_ in range(3):
            diff[_BLK_DST, cmp_] = dD[_BLK_SRC3 + cmp_]
        outs.append((ind, dist, diff))

    indices = np.concatenate([o[0] for o in outs], axis=1)
    dist = np.concatenate([o[1] for o in outs], axis=0)
    diff = np.concatenate([o[2] for o in outs], axis=0)
    if _trace:
        kernel.last_exec_time_ns = res.exec_time_ns
        kernel.last_results = res
    return indices, dist, diff


# revision 22
# speedup vs baseline: 3.1765x; 1.0359x over previous
"""AllPairs triu kernel for Trainium2 (8 NeuronCores, one molecule per core).

Computes, for each molecule of N=2048 atoms, all upper-triangle pairs:
  indices [2, P] int32 (flat atom ids, -1 where invalid)
  dist    [P]    f32   (||ri - rj|| where valid else 0)
  diff    [P, 3] f32   (ri - rj where valid else 0)
with P = B * N*(N-1)/2, valid = both atoms real and dist <= 5.2.

Strategy: data-parallel over the batch axis (1 molecule per core). Each core
computes 128-row i-tiles against the trimmed j-range [128t, N), then writes
the packed triangular output directly with indirect-scatter DMAs:
  - main chunk per row: constant length C_t = 1920-128t starting at the row's
    packed base (diagonal source access pattern, row p starts at column p+1);
  - one backward window per tile: each row's last 127 elements, which are
    always the columns j in [1921, 2048) (clean access pattern); overlap with
    the main chunk rewrites identical bytes, which is benign.
The i-tile t=15 (row lengths < 128, total 8128 pairs = 0.4% of the output)
is emitted as a small dense block and packed on the host.
The dist<=cutoff mask is computed exactly as sqrt-compare via an equivalent
dist^2 threshold, so masks match the IEEE reference bit-for-bit.
"""
import sys

sys.path.insert(0, "/opt/trn_rl_repo")

import numpy as np

N = 2048
B = 8
NT = 16                  # i-tiles per molecule
PM = N * (N - 1) // 2    # pairs per molecule = 2096128
CUTOFF = 5.2
NCOLS = 32               # offset-table columns per tensor


def _exact_d2_threshold():
    """Largest f32 u with np.sqrt(f32(u)) <= f32(5.2).

    sqrt is monotone and IEEE-correctly-rounded on both CPU (reference) and in
    this comparison, so (d2 <= U) == (sqrt(d2) <= 5.2f) for every f32 d2.
    """
    c = np.float32(CUTOFF)
    lo_bits = (c * c).view(np.uint32)
    while np.sqrt(lo_bits.view(np.float32)) > c:
        lo_bits = np.uint32(lo_bits - 1)
    hi_bits = np.uint32(lo_bits + 1000)
    assert np.sqrt(hi_bits.view(np.float32)) > c
    lo_bits, hi_bits = int(lo_bits), int(hi_bits)
    while hi_bits - lo_bits > 1:
        mid = (lo_bits + hi_bits) // 2
        if np.sqrt(np.uint32(mid).view(np.float32)) <= c:
            lo_bits = mid
        else:
            hi_bits = mid
    return float(np.uint32(lo_bits).view(np.float32))


U_D2 = _exact_d2_threshold()


def _base(i):
    """Packed offset of row i in the per-molecule triu output."""
    return 2047 * i - (i * (i - 1)) // 2


def _build_offset_table():
    """[128, NCOLS] int64: cols 0..14 mains(t): row p covers columns
    j in [128(t+1), 2048), i.e. packed [base(i) + 127 - p, base(i) + L)."""
    tab = np.zeros((128, NCOLS), dtype=np.int64)
    p = np.arange(128)
    for t in range(15):
        i = 128 * t + p
        tab[:, t] = _base(i) + 127 - p
    return tab


_OFFSET_TABLE = _build_offset_table()


def _tensor_tables():
    t = _OFFSET_TABLE
    mk = lambda s, o: (s * t + o).astype(np.int32)
    return {"tabsall": np.concatenate(
        [mk(1, 0), mk(1, 0), mk(1, PM), mk(3, 0)], axis=1)}


# ----------------------------------------------------------------------------
# Bass graph
# ----------------------------------------------------------------------------

_GRAPH = None


def _build_graph():
    import concourse.bass as bass
    import concourse.bacc as bacc
    import concourse.mybir as mybir
    from concourse.tile import TileContext

    OP = mybir.AluOpType
    AF = mybir.ActivationFunctionType
    f32 = mybir.dt.float32
    i32 = mybir.dt.int32
    i16 = mybir.dt.int16

    nc = bacc.Bacc("TRN2", debug=False, num_devices=B, detect_race_conditions=False,
                   dynamic_dma_scratch_size=8192)

    negb3 = nc.dram_tensor("negb3", [128, 3 * N], f32, kind="ExternalInput")
    jrow = nc.dram_tensor("jrow", [128, N], f32, kind="ExternalInput")
    # combined small input: cols [0,64) = xyzi+ibias (f32 bits), [64,192) = tables
    xibtab = nc.dram_tensor("xibtab", [128, 4 * NT + 4 * NCOLS], i32, kind="ExternalInput")

    indices_o = nc.dram_tensor("indices", [2, PM], i16, kind="ExternalOutput")
    dist_o = nc.dram_tensor("dist", [PM], f32, kind="ExternalOutput")
    diff_o = nc.dram_tensor("diff", [PM, 3], f32, kind="ExternalOutput")
    # per-tile diagonal 128x128 blocks, packed on the host
    blkd = nc.dram_tensor("blkd", [NT * 128, 128], f32, kind="ExternalOutput")
    blki0 = nc.dram_tensor("blki0", [NT * 128, 128], i16, kind="ExternalOutput")
    blki1 = nc.dram_tensor("blki1", [NT * 128, 128], i16, kind="ExternalOutput")
    blkD = nc.dram_tensor("blkD", [NT * 128, 384], f32, kind="ExternalOutput")

    ind_flat = indices_o[:].flatten().unsqueeze(1)
    dist_flat = dist_o[:].unsqueeze(1)
    diff_flat = diff_o[:].flatten().unsqueeze(1)

    out_dma_names = set()

    def _strip_outdma_deps(bi):
        """Output DMAs overlap only where bytes are identical; drop the WAW
        serialization Tile would impose so they pipeline."""
        ins = bi.ins
        for d in list(ins.sync_dependency_names()):
            if d in out_dma_names:
                ins.try_remove_dependency(d)
        for d in list(ins.nosync_dependency_names()):
            if d in out_dma_names:
                ins.try_remove_dependency(d)
        out_dma_names.add(ins.name)
        return bi

    with TileContext(nc) as tc:
        tc.race_detector_enabled = False
        with (
            tc.tile_pool(name="persist", bufs=1) as pp,
            tc.tile_pool(name="outs", bufs=2) as po,
            tc.tile_pool(name="dxyzp", bufs=2) as pd,
            tc.tile_pool(name="sqa", bufs=2) as ps,
            tc.tile_pool(name="sqbc", bufs=1) as psc,
        ):
            t_negb3 = pp.tile([128, 3 * N], f32, tag="negb3")
            for c in range(3):
                nc.sync.dma_start(out=t_negb3[:, c * N : (c + 1) * N],
                                  in_=negb3[:, c * N : (c + 1) * N])
            t_jrow = pp.tile([128, N], f32, tag="jrow")
            nc.sync.dma_start(out=t_jrow[:], in_=jrow[:])
            t_comb = pp.tile([128, 4 * NT + 4 * NCOLS], i32, tag="xibtab")
            nc.sync.dma_start(out=t_comb[:], in_=xibtab[:])
            TABW = 4 * NT + 4 * NCOLS
            TAB0 = 4 * NT

            def scatter(dst_flat, tab_off, src_tile, src_off, src_dims):
                toff = bass.AP(t_comb[:].tensor, TAB0 + tab_off, [[TABW, 128], [1, 1]])
                src = bass.AP(src_tile[:].tensor, src_off, src_dims)
                bi = nc.gpsimd.indirect_dma_start(
                    out=dst_flat,
                    out_offset=bass.IndirectOffsetOnAxis(ap=toff, axis=0),
                    in_=src,
                    in_offset=None,
                )
                return _strip_outdma_deps(bi)

            def phase_a(t):
                W = N - 128 * t
                t_diff = po.tile([128, 3 * N], f32, tag="diff", name=f"diff{t}")
                t_dist = po.tile([128, N], f32, tag="dist", name=f"dist{t}")
                t_idx0 = po.tile([128, N], i16, tag="idx0", name=f"idx0_{t}")
                t_idx1 = po.tile([128, N], i16, tag="idx1", name=f"idx1_{t}")
                dxyz = pd.tile([128, 3 * N], f32, tag="dxyz", name=f"dxyz{t}")
                sq_a = ps.tile([128, N], f32, tag="sq_a", name=f"sqa{t}")
                sq_b = psc.tile([128, N], f32, tag="sq_b", name=f"sqb{t}")
                sq_c = psc.tile([128, N], f32, tag="sq_c", name=f"sqc{t}")

                # ACT: diff components, contiguous blocks (exact IEEE adds)
                for c in range(3):
                    nc.scalar.activation(
                        out=dxyz[:, c * N : c * N + W],
                        in_=t_negb3[:, c * N + 128 * t : (c + 1) * N],
                        func=AF.Identity,
                        bias=t_comb[:, 3 * t + c : 3 * t + c + 1].bitcast(f32),
                        scale=1.0,
                    )
                # ACT: squares, contiguous (bit-exact)
                nc.scalar.activation(out=sq_a[:, :W], in_=dxyz[:, 0:W], func=AF.Square)
                nc.scalar.activation(out=sq_b[:, :W], in_=dxyz[:, N : N + W], func=AF.Square)
                nc.scalar.activation(out=sq_c[:, :W], in_=dxyz[:, 2 * N : 2 * N + W], func=AF.Square)
                # DVE: d2 = sqx + sqy + sqz ; mask ; d2m ; masked interleave
                nc.vector.tensor_tensor(out=sq_a[:, :W], in0=sq_a[:, :W], in1=sq_b[:, :W], op=OP.add)
                nc.vector.tensor_tensor(out=sq_a[:, :W], in0=sq_a[:, :W], in1=sq_c[:, :W], op=OP.add)
                nc.vector.tensor_scalar(out=sq_b[:, :W], in0=sq_a[:, :W],
                                        scalar1=U_D2, scalar2=None, op0=OP.is_le)
                nc.vector.tensor_tensor(out=sq_a[:, :W], in0=sq_b[:, :W], in1=sq_a[:, :W], op=OP.mult)
                # diff[:, 3j+c] = dxyz[c-block][j] * mask[j]  (one op, [W,3] view)
                dsrc = bass.AP(dxyz[:].tensor, 0, [[3 * N, 128], [1, W], [N, 3]])
                m3 = bass.AP(sq_b[:].tensor, 0, [[N, 128], [1, W], [0, 3]])
                dout = bass.AP(t_diff[:].tensor, 0, [[3 * N, 128], [3, W], [1, 3]])
                nc.vector.tensor_tensor(out=dout, in0=dsrc, in1=m3, op=OP.mult)
                nc.scalar.activation(out=t_idx0[:, :W], in_=sq_b[:, :W], func=AF.Copy,
                                     bias=-1.0, scale=t_comb[:, 48 + t : 48 + t + 1].bitcast(f32))
                nc.vector.scalar_tensor_tensor(out=t_idx1[:, :W],
                                               in0=t_jrow[:, 128 * t :], scalar=1.0,
                                               in1=sq_b[:, :W], op0=OP.mult, op1=OP.mult)
                nc.scalar.activation(out=t_idx1[:, :W], in_=t_idx1[:, :W], func=AF.Copy,
                                     bias=-1.0, scale=1.0)
                return dict(t=t, W=W, diff=t_diff, dist=t_dist, idx0=t_idx0,
                            idx1=t_idx1, sq_a=sq_a, sq_b=sq_b)

            def phase_b(st):
                t, W = st["t"], st["W"]
                C = 1920 - 128 * t
                t_diff, t_dist = st["diff"], st["dist"]
                t_idx0, t_idx1 = st["idx0"], st["idx1"]
                sq_a, sq_b = st["sq_a"], st["sq_b"]
                nc.scalar.activation(out=t_dist[:, :W], in_=sq_a[:, :W], func=AF.Sqrt)
                if t < 15:
                    # mains: every row sources local cols [128, W);
                    # dest = base(i) + 127 - p, length C = W - 128
                    scatter(dist_flat, t, t_dist, 128, [[N, 128], [1, C]])
                    scatter(ind_flat, NCOLS + t, t_idx0, 128, [[N, 128], [1, C]])
                    scatter(ind_flat, 2 * NCOLS + t, t_idx1, 128, [[N, 128], [1, C]])
                    scatter(diff_flat, 3 * NCOLS + t, t_diff, 384, [[3 * N, 128], [1, 3 * C]])
                # diagonal block dump (cols [0, 128)), host-packed
                for dst, tile_, w in ((blkd, t_dist, 128), (blki0, t_idx0, 128),
                                      (blki1, t_idx1, 128), (blkD, t_diff, 384)):
                    _strip_outdma_deps(nc.sync.dma_start(
                        out=dst[128 * t : 128 * (t + 1), :], in_=tile_[:, :w]))

            # software pipeline: phase B lags one tile behind phase A
            prev = phase_a(0)
            for t in range(1, NT):
                cur = phase_a(t)
                phase_b(prev)
                prev = cur
            phase_b(prev)

    nc.compile()
    return nc


def _get_graph():
    global _GRAPH
    if _GRAPH is None:
        _GRAPH = _build_graph()
    return _GRAPH


# ----------------------------------------------------------------------------
# Host glue
# ----------------------------------------------------------------------------

# diagonal-block host packing: packed positions and dense-block sources
_BLK_P, _BLK_C = np.triu_indices(128, 1)
_BLK_DST = np.concatenate([
    _base(128 * t + _BLK_P) + (_BLK_C - _BLK_P - 1) for t in range(NT)
]).astype(np.int64)
_BLK_SRC = np.concatenate([
    (128 * t + _BLK_P) * 128 + _BLK_C for t in range(NT)
]).astype(np.int64)
_BLK_SRC3 = np.concatenate([
    (128 * t + _BLK_P) * 384 + 3 * _BLK_C for t in range(NT)
]).astype(np.int64)


def _prep_core_inputs(b, coords_mod, tables):
    cb = coords_mod[b]  # [N, 3] f32
    negrow = (-cb.T).reshape(1, 3 * N)
    negb3 = np.ascontiguousarray(np.broadcast_to(negrow, (128, 3 * N)), dtype=np.float32)
    jr = (np.arange(N, dtype=np.float32) + np.float32(b * N + 1)).reshape(1, N)
    jrow = np.ascontiguousarray(np.broadcast_to(jr, (128, N)), dtype=np.float32)
    xyzi = cb.reshape(NT, 128, 3).transpose(1, 0, 2).reshape(128, 3 * NT)
    p = np.arange(128, dtype=np.float32).reshape(128, 1)
    tgrid = np.arange(NT, dtype=np.float32).reshape(1, NT)
    ibias = (128.0 * tgrid + p) + np.float32(b * N + 1)
    xibias = np.concatenate([xyzi, ibias], axis=1).astype(np.float32)
    xibtab = np.ascontiguousarray(np.concatenate(
        [xibias.view(np.int32), tables["tabsall"]], axis=1))
    return {"negb3": negb3, "jrow": jrow, "xibtab": xibtab}


def kernel(species, coords, _trace=False):
    from concourse.bass_utils import run_bass_kernel_spmd

    species = np.asarray(species)
    coords = np.ascontiguousarray(np.asarray(coords, dtype=np.float32))
    assert coords.shape == (B, N, 3), coords.shape

    coords_mod = coords
    dummy = species == -1  # [B, N]
    if dummy.any():
        far = (1e6 * (np.arange(N, dtype=np.float32) + 1.0))[None, :, None]
        far = np.broadcast_to(far, coords.shape)
        coords_mod = np.where(dummy[:, :, None], far, coords).astype(np.float32)

    tables = _tensor_tables()
    nc = _get_graph()
    in_maps = [_prep_core_inputs(b, coords_mod, tables) for b in range(B)]
    res = run_bass_kernel_spmd(nc, in_maps, core_ids=list(range(B)), trace=_trace)

    outs = []
    for c in range(B):
        r = res.results[c]
        ind = np.array(r["indices"])
        dist = np.array(r["dist"])
        diff = np.array(r["diff"]).reshape(PM, 3)
        # host-pack the 16 diagonal-block triangles (6.2% of pairs)
        ind[0, _BLK_DST] = r["blki0"].reshape(-1)[_BLK_SRC]
        ind[1, _BLK_DST] = r["blki1"].reshape(-1)[_BLK_SRC]
        dist[_BLK_DST] = r["blkd"].reshape(-1)[_BLK_SRC]
        dD = r["blkD"].reshape(-1)
        for cmp_ in range(3):
            diff[_BLK_DST, cmp_] = dD[_BLK_SRC3 + cmp_]
        outs.append((ind, dist, diff))

    indices = np.concatenate([o[0] for o in outs], axis=1).astype(np.int32)
    dist = np.concatenate([o[1] for o in outs], axis=0)
    diff = np.concatenate([o[2] for o in outs], axis=0)
    if _trace:
        kernel.last_exec_time_ns = res.exec_time_ns
        kernel.last_results = res
    return indices, dist, diff


# revision 23
# speedup vs baseline: 3.4130x; 1.0744x over previous
"""AllPairs triu kernel for Trainium2 (8 NeuronCores, one molecule per core).

Computes, for each molecule of N=2048 atoms, all upper-triangle pairs:
  indices [2, P] int32 (flat atom ids, -1 where invalid)
  dist    [P]    f32   (||ri - rj|| where valid else 0)
  diff    [P, 3] f32   (ri - rj where valid else 0)
with P = B * N*(N-1)/2, valid = both atoms real and dist <= 5.2.

Strategy: data-parallel over the batch axis (1 molecule per core). Each core
computes 128-row i-tiles against the trimmed j-range [128t, N), then writes
the packed triangular output directly with indirect-scatter DMAs:
  - main chunk per row: constant length C_t = 1920-128t starting at the row's
    packed base (diagonal source access pattern, row p starts at column p+1);
  - one backward window per tile: each row's last 127 elements, which are
    always the columns j in [1921, 2048) (clean access pattern); overlap with
    the main chunk rewrites identical bytes, which is benign.
The i-tile t=15 (row lengths < 128, total 8128 pairs = 0.4% of the output)
is emitted as a small dense block and packed on the host.
The dist<=cutoff mask is computed exactly as sqrt-compare via an equivalent
dist^2 threshold, so masks match the IEEE reference bit-for-bit.
"""
import sys

sys.path.insert(0, "/opt/trn_rl_repo")

import numpy as np

N = 2048
B = 8
NT = 16                  # i-tiles per molecule
PM = N * (N - 1) // 2    # pairs per molecule = 2096128
CUTOFF = 5.2
NCOLS = 32               # offset-table columns per tensor


def _exact_d2_threshold():
    """Largest f32 u with np.sqrt(f32(u)) <= f32(5.2).

    sqrt is monotone and IEEE-correctly-rounded on both CPU (reference) and in
    this comparison, so (d2 <= U) == (sqrt(d2) <= 5.2f) for every f32 d2.
    """
    c = np.float32(CUTOFF)
    lo_bits = (c * c).view(np.uint32)
    while np.sqrt(lo_bits.view(np.float32)) > c:
        lo_bits = np.uint32(lo_bits - 1)
    hi_bits = np.uint32(lo_bits + 1000)
    assert np.sqrt(hi_bits.view(np.float32)) > c
    lo_bits, hi_bits = int(lo_bits), int(hi_bits)
    while hi_bits - lo_bits > 1:
        mid = (lo_bits + hi_bits) // 2
        if np.sqrt(np.uint32(mid).view(np.float32)) <= c:
            lo_bits = mid
        else:
            hi_bits = mid
    return float(np.uint32(lo_bits).view(np.float32))


U_D2 = _exact_d2_threshold()


def _base(i):
    """Packed offset of row i in the per-molecule triu output."""
    return 2047 * i - (i * (i - 1)) // 2


def _build_offset_table():
    """[128, NCOLS] int64: cols 0..14 mains(t): row p covers columns
    j in [128(t+1), 2048), i.e. packed [base(i) + 127 - p, base(i) + L)."""
    tab = np.zeros((128, NCOLS), dtype=np.int64)
    p = np.arange(128)
    for t in range(15):
        i = 128 * t + p
        tab[:, t] = _base(i) + 127 - p
    return tab


_OFFSET_TABLE = _build_offset_table()


def _tensor_tables():
    t = _OFFSET_TABLE
    mk = lambda s, o: (s * t + o).astype(np.int32)
    return {"tabsall": np.concatenate(
        [mk(1, 0), mk(1, 0), mk(1, PM), mk(3, 0)], axis=1)}


# ----------------------------------------------------------------------------
# Bass graph
# ----------------------------------------------------------------------------

_GRAPH = None


def _build_graph():
    import concourse.bass as bass
    import concourse.bacc as bacc
    import concourse.mybir as mybir
    from concourse.tile import TileContext

    OP = mybir.AluOpType
    AF = mybir.ActivationFunctionType
    f32 = mybir.dt.float32
    i32 = mybir.dt.int32
    i16 = mybir.dt.int16

    nc = bacc.Bacc("TRN2", debug=False, num_devices=B, detect_race_conditions=False,
                   dynamic_dma_scratch_size=8192)

    negb3 = nc.dram_tensor("negb3", [128, 3 * N], f32, kind="ExternalInput")
    jrow = nc.dram_tensor("jrow", [128, N], f32, kind="ExternalInput")
    # combined small input: cols [0,64) = xyzi+ibias (f32 bits), [64,192) = tables
    xibtab = nc.dram_tensor("xibtab", [128, 4 * NT + 4 * NCOLS], i32, kind="ExternalInput")

    indices_o = nc.dram_tensor("indices", [2, PM], i16, kind="ExternalOutput")
    dist_o = nc.dram_tensor("dist", [PM], f32, kind="ExternalOutput")
    diff_o = nc.dram_tensor("diff", [PM, 3], f32, kind="ExternalOutput")
    # per-tile diagonal 128x128 blocks, packed on the host
    blkd = nc.dram_tensor("blkd", [NT * 128, 128], f32, kind="ExternalOutput")
    blki0 = nc.dram_tensor("blki0", [NT * 128, 128], i16, kind="ExternalOutput")
    blki1 = nc.dram_tensor("blki1", [NT * 128, 128], i16, kind="ExternalOutput")
    blkD = nc.dram_tensor("blkD", [NT * 128, 384], f32, kind="ExternalOutput")

    ind_flat = indices_o[:].flatten().unsqueeze(1)
    dist_flat = dist_o[:].unsqueeze(1)
    diff_flat = diff_o[:].flatten().unsqueeze(1)

    out_dma_names = set()

    def _strip_outdma_deps(bi):
        """Output DMAs overlap only where bytes are identical; drop the WAW
        serialization Tile would impose so they pipeline."""
        ins = bi.ins
        for d in list(ins.sync_dependency_names()):
            if d in out_dma_names:
                ins.try_remove_dependency(d)
        for d in list(ins.nosync_dependency_names()):
            if d in out_dma_names:
                ins.try_remove_dependency(d)
        out_dma_names.add(ins.name)
        return bi

    with TileContext(nc) as tc:
        tc.race_detector_enabled = False
        with (
            tc.tile_pool(name="persist", bufs=1) as pp,
            tc.tile_pool(name="outs", bufs=2) as po,
            tc.tile_pool(name="dxyzp", bufs=2) as pd,
            tc.tile_pool(name="sqa", bufs=2) as ps,
            tc.tile_pool(name="sqbc", bufs=1) as psc,
        ):
            t_negb3 = pp.tile([128, 3 * N], f32, tag="negb3")
            for c in range(3):
                nc.sync.dma_start(out=t_negb3[:, c * N : (c + 1) * N],
                                  in_=negb3[:, c * N : (c + 1) * N])
            t_jrow = pp.tile([128, N], f32, tag="jrow")
            nc.sync.dma_start(out=t_jrow[:], in_=jrow[:])
            t_comb = pp.tile([128, 4 * NT + 4 * NCOLS], i32, tag="xibtab")
            nc.sync.dma_start(out=t_comb[:], in_=xibtab[:])
            TABW = 4 * NT + 4 * NCOLS
            TAB0 = 4 * NT

            def scatter(dst_flat, tab_off, src_tile, src_off, src_dims):
                toff = bass.AP(t_comb[:].tensor, TAB0 + tab_off, [[TABW, 128], [1, 1]])
                src = bass.AP(src_tile[:].tensor, src_off, src_dims)
                bi = nc.gpsimd.indirect_dma_start(
                    out=dst_flat,
                    out_offset=bass.IndirectOffsetOnAxis(ap=toff, axis=0),
                    in_=src,
                    in_offset=None,
                )
                return _strip_outdma_deps(bi)

            def phase_a(t):
                W = N - 128 * t
                t_diff = po.tile([128, 3 * N], f32, tag="diff", name=f"diff{t}")
                t_dist = po.tile([128, N], f32, tag="dist", name=f"dist{t}")
                t_idx0 = po.tile([128, N], i16, tag="idx0", name=f"idx0_{t}")
                t_idx1 = po.tile([128, N], i16, tag="idx1", name=f"idx1_{t}")
                dxyz = pd.tile([128, 3 * N], f32, tag="dxyz", name=f"dxyz{t}")
                sq_a = ps.tile([128, N], f32, tag="sq_a", name=f"sqa{t}")
                sq_b = psc.tile([128, N], f32, tag="sq_b", name=f"sqb{t}")
                sq_c = psc.tile([128, N], f32, tag="sq_c", name=f"sqc{t}")

                # ACT: diff components, contiguous blocks (exact IEEE adds)
                for c in range(3):
                    nc.scalar.activation(
                        out=dxyz[:, c * N : c * N + W],
                        in_=t_negb3[:, c * N + 128 * t : (c + 1) * N],
                        func=AF.Identity,
                        bias=t_comb[:, 3 * t + c : 3 * t + c + 1].bitcast(f32),
                        scale=1.0,
                    )
                # ACT: squares, contiguous (bit-exact)
                nc.scalar.activation(out=sq_a[:, :W], in_=dxyz[:, 0:W], func=AF.Square)
                nc.scalar.activation(out=sq_b[:, :W], in_=dxyz[:, N : N + W], func=AF.Square)
                nc.scalar.activation(out=sq_c[:, :W], in_=dxyz[:, 2 * N : 2 * N + W], func=AF.Square)
                # DVE: d2 = sqx + sqy + sqz ; mask ; d2m ; masked interleave
                nc.vector.tensor_tensor(out=sq_a[:, :W], in0=sq_a[:, :W], in1=sq_b[:, :W], op=OP.add)
                nc.vector.tensor_tensor(out=sq_a[:, :W], in0=sq_a[:, :W], in1=sq_c[:, :W], op=OP.add)
                nc.vector.tensor_scalar(out=sq_b[:, :W], in0=sq_a[:, :W],
                                        scalar1=U_D2, scalar2=None, op0=OP.is_le)
                # diff[:, 3j+c] = dxyz[c-block][j] * mask[j]  (one op, [W,3] view)
                dsrc = bass.AP(dxyz[:].tensor, 0, [[3 * N, 128], [1, W], [N, 3]])
                m3 = bass.AP(sq_b[:].tensor, 0, [[N, 128], [1, W], [0, 3]])
                dout = bass.AP(t_diff[:].tensor, 0, [[3 * N, 128], [3, W], [1, 3]])
                nc.vector.tensor_tensor(out=dout, in0=dsrc, in1=m3, op=OP.mult)
                nc.scalar.activation(out=t_idx0[:, :W], in_=sq_b[:, :W], func=AF.Copy,
                                     bias=-1.0, scale=t_comb[:, 48 + t : 48 + t + 1].bitcast(f32))
                nc.vector.scalar_tensor_tensor(out=t_idx1[:, :W],
                                               in0=t_jrow[:, 128 * t :], scalar=1.0,
                                               in1=sq_b[:, :W], op0=OP.mult, op1=OP.mult)
                nc.scalar.activation(out=t_idx1[:, :W], in_=t_idx1[:, :W], func=AF.Copy,
                                     bias=-1.0, scale=1.0)
                return dict(t=t, W=W, diff=t_diff, dist=t_dist, idx0=t_idx0,
                            idx1=t_idx1, sq_a=sq_a, sq_b=sq_b)

            def phase_b(st):
                t, W = st["t"], st["W"]
                C = 1920 - 128 * t
                t_diff, t_dist = st["diff"], st["dist"]
                t_idx0, t_idx1 = st["idx0"], st["idx1"]
                sq_a, sq_b = st["sq_a"], st["sq_b"]
                nc.scalar.activation(out=t_dist[:, :W], in_=sq_a[:, :W], func=AF.Sqrt)
                nc.vector.scalar_tensor_tensor(out=t_dist[:, :W],
                                               in0=sq_a[:, :W], scalar=U_D2,
                                               in1=t_dist[:, :W], op0=OP.is_le, op1=OP.mult)
                if t < 15:
                    # mains: every row sources local cols [128, W);
                    # dest = base(i) + 127 - p, length C = W - 128
                    scatter(dist_flat, t, t_dist, 128, [[N, 128], [1, C]])
                    scatter(ind_flat, NCOLS + t, t_idx0, 128, [[N, 128], [1, C]])
                    scatter(ind_flat, 2 * NCOLS + t, t_idx1, 128, [[N, 128], [1, C]])
                    scatter(diff_flat, 3 * NCOLS + t, t_diff, 384, [[3 * N, 128], [1, 3 * C]])
                # diagonal block dump (cols [0, 128)), host-packed
                for dst, tile_, w in ((blkd, t_dist, 128), (blki0, t_idx0, 128),
                                      (blki1, t_idx1, 128), (blkD, t_diff, 384)):
                    _strip_outdma_deps(nc.sync.dma_start(
                        out=dst[128 * t : 128 * (t + 1), :], in_=tile_[:, :w]))

            # software pipeline: phase B lags one tile behind phase A
            prev = phase_a(0)
            for t in range(1, NT):
                cur = phase_a(t)
                phase_b(prev)
                prev = cur
            phase_b(prev)

    nc.compile()
    return nc


def _get_graph():
    global _GRAPH
    if _GRAPH is None:
        _GRAPH = _build_graph()
    return _GRAPH


# ----------------------------------------------------------------------------
# Host glue
# ----------------------------------------------------------------------------

# diagonal-block host packing: packed positions and dense-block sources
_BLK_P, _BLK_C = np.triu_indices(128, 1)
_BLK_DST = np.concatenate([
    _base(128 * t + _BLK_P) + (_BLK_C - _BLK_P - 1) for t in range(NT)
]).astype(np.int64)
_BLK_SRC = np.concatenate([
    (128 * t + _BLK_P) * 128 + _BLK_C for t in range(NT)
]).astype(np.int64)
_BLK_SRC3 = np.concatenate([
    (128 * t + _BLK_P) * 384 + 3 * _BLK_C for t in range(NT)
]).astype(np.int64)


def _prep_core_inputs(b, coords_mod, tables):
    cb = coords_mod[b]  # [N, 3] f32
    negrow = (-cb.T).reshape(1, 3 * N)
    negb3 = np.ascontiguousarray(np.broadcast_to(negrow, (128, 3 * N)), dtype=np.float32)
    jr = (np.arange(N, dtype=np.float32) + np.float32(b * N + 1)).reshape(1, N)
    jrow = np.ascontiguousarray(np.broadcast_to(jr, (128, N)), dtype=np.float32)
    xyzi = cb.reshape(NT, 128, 3).transpose(1, 0, 2).reshape(128, 3 * NT)
    p = np.arange(128, dtype=np.float32).reshape(128, 1)
    tgrid = np.arange(NT, dtype=np.float32).reshape(1, NT)
    ibias = (128.0 * tgrid + p) + np.float32(b * N + 1)
    xibias = np.concatenate([xyzi, ibias], axis=1).astype(np.float32)
    xibtab = np.ascontiguousarray(np.concatenate(
        [xibias.view(np.int32), tables["tabsall"]], axis=1))
    return {"negb3": negb3, "jrow": jrow, "xibtab": xibtab}


def kernel(species, coords, _trace=False):
    from concourse.bass_utils import run_bass_kernel_spmd

    species = np.asarray(species)
    coords = np.ascontiguousarray(np.asarray(coords, dtype=np.float32))
    assert coords.shape == (B, N, 3), coords.shape

    coords_mod = coords
    dummy = species == -1  # [B, N]
    if dummy.any():
        far = (1e6 * (np.arange(N, dtype=np.float32) + 1.0))[None, :, None]
        far = np.broadcast_to(far, coords.shape)
        coords_mod = np.where(dummy[:, :, None], far, coords).astype(np.float32)

    tables = _tensor_tables()
    nc = _get_graph()
    in_maps = [_prep_core_inputs(b, coords_mod, tables) for b in range(B)]
    res = run_bass_kernel_spmd(nc, in_maps, core_ids=list(range(B)), trace=_trace)

    outs = []
    for c in range(B):
        r = res.results[c]
        ind = np.array(r["indices"])
        dist = np.array(r["dist"])
        diff = np.array(r["diff"]).reshape(PM, 3)
        # host-pack the 16 diagonal-block triangles (6.2% of pairs)
        ind[0, _BLK_DST] = r["blki0"].reshape(-1)[_BLK_SRC]
        ind[1, _BLK_DST] = r["blki1"].reshape(-1)[_BLK_SRC]
        dist[_BLK_DST] = r["blkd"].reshape(-1)[_BLK_SRC]
        dD = r["blkD"].reshape(-1)
        for cmp_ in range(3):
            diff[_BLK_DST, cmp_] = dD[_BLK_SRC3 + cmp_]
        outs.append((ind, dist, diff))

    indices = np.concatenate([o[0] for o in outs], axis=1).astype(np.int32)
    dist = np.concatenate([o[1] for o in outs], axis=0)
    diff = np.concatenate([o[2] for o in outs], axis=0)
    if _trace:
        kernel.last_exec_time_ns = res.exec_time_ns
        kernel.last_results = res
    return indices, dist, diff
